# revision 10
# baseline (speedup 1.0000x reference)
"""Trainium2 Bass kernel for SSTransformer channel-attention block.

Sharding: 8 cores; core c handles sample c//2, row-half c%2 (128 of 256 rows).
Per core, one fused Bass program computes:
  - fused qkv 1x1 conv + depthwise 3x3 (6 PSUM-accumulated matmuls per row over
    a duplicated row-shifted x layout [x; x@+1row] on 128 partitions)
  - Gram matrix of [q;k] (per-head q.k^T dots + squared norms) via PE transpose
  - tiny cross-core AllReduce (core pairs) of the 128x128 Gram
  - on-chip softmax + rel-bias fold into the projection weights
  - (attn@v + proj) and the positional branch dw3x3 -> GELU -> dw3x3, both
    accumulated in PSUM, + proj bias, DMA out.
"""

import sys

sys.path.insert(0, "/opt/trn_rl_repo")

import numpy as np
import ml_dtypes

HEADS = 8
C = 64
CH = 8
B = 4
H = 256
WIMG = 256
WP = 258          # padded row stride (1 zero col each side, shared)
NCORES = 8
ROWS = 128        # output rows per core
VR0 = -2          # v2 first row (relative to slab start)
VROWS = 132       # v2 rows: -2 .. 129
XTOP = 3          # zero-pad rows above image in padded x
XBOT = 4          # below
XROWS = 135       # x rows per core slab: r0-3 .. r0+131
CW = 22           # conv window height (v2 rows per window); 6 windows
NCW = 6
PW = 16           # pe window output rows; 8 windows
NPW = 8
EPS = 1e-12

_BASES = [(-1, -1), (-1, 0), (-1, 1), (1, -1), (1, 0), (1, 1)]  # (dy,dx) of base offsets

_cache = {}


def _build_conv_weights(qkv_w, dw_w):
    """12 lhsT matrices [128,128] f32, flattened to [128, 12*128].

    Matmul m (m<6: pass1 -> [q;k]; m>=6: pass2 -> [v; v@WP]) with base offset
    delta = dy*WP+dx reads partition p<64: x[p] at j+delta, p>=64: x[p-64] at
    j+delta+WP.
    """
    w1 = qkv_w[:, :, 0, 0]  # [192 oc, 64 ic]

    def tapw(oc0, dy, dx):
        # returns [64 ic, 64 oc] = lhsT block for tap (dy,dx), out channels oc0..oc0+64
        blk = w1[oc0 : oc0 + 64]  # [64 oc, 64 ic]
        d = dw_w[oc0 : oc0 + 64, 0, dy + 1, dx + 1]  # [64]
        return (blk * d[:, None]).T.astype(np.float32)

    mats = []
    for dy, dx in _BASES:
        m = np.zeros((128, 128), np.float32)
        # pass1: cols 0:64 = q (oc0=0), 64:128 = k (oc0=64)
        for ci, oc0 in ((0, 0), (64, 64)):
            m[0:64, ci : ci + 64] = tapw(oc0, dy, dx)          # A rows: tap (dy,dx)
            if dy == -1:
                m[64:128, ci : ci + 64] = tapw(oc0, 0, dx)     # B rows: tap (0,dx)
        mats.append(m)
    for dy, dx in _BASES:
        m = np.zeros((128, 128), np.float32)
        # pass2: cols 0:64 = v (oc0=128), 64:128 = v@WP
        m[0:64, 0:64] = tapw(128, dy, dx)
        if dy == -1:
            m[64:128, 0:64] = tapw(128, 0, dx)
            m[64:128, 64:128] = tapw(128, dy, dx)
        else:
            m[0:64, 64:128] = tapw(128, 0, dx)
            m[64:128, 64:128] = tapw(128, dy, dx)
        mats.append(m)
    return np.concatenate(mats, axis=1)  # [128, 12*128]


def _build_dw_weights(pos_w, out_shifted):
    """6 lhsT diag-block matrices for a depthwise 3x3 over [t; t@WP] input.

    out_shifted: if True output is [o; o@WP] (M=128), else just o (M=64).
    Returns [128, 6*M] float32 (cast to bf16 by caller).
    """
    M = 128 if out_shifted else 64

    def dtap(dy, dx):
        return np.diag(pos_w[:, 0, dy + 1, dx + 1]).astype(np.float32)

    mats = []
    for dy, dx in _BASES:
        m = np.zeros((128, M), np.float32)
        m[0:64, 0:64] = dtap(dy, dx)
        if dy == -1:
            m[64:128, 0:64] = dtap(0, dx)
            if out_shifted:
                m[64:128, 64:128] = dtap(dy, dx)
        else:
            if out_shifted:
                m[0:64, 64:128] = dtap(0, dx)
                m[64:128, 64:128] = dtap(dy, dx)
    # NOTE: for dy==+1 and not out_shifted, only A rows are used.
        mats.append(m)
    return np.concatenate(mats, axis=1)


def _build_program(debug=False):
    import concourse.bass as bass
    import concourse.bacc as bacc
    import concourse.mybir as mybir
    from concourse import tile

    dt = mybir.dt
    AF = mybir.ActivationFunctionType
    ALU = mybir.AluOpType
    f32, bf16 = dt.float32, dt.bfloat16

    nc = bacc.Bacc("TRN2", target_bir_lowering=False, debug=False, num_devices=NCORES)

    xp_d = nc.dram_tensor("xp", [C, XROWS * WP], bf16, kind="ExternalInput")
    cw_d = nc.dram_tensor("cw", [128, 12 * 128], bf16, kind="ExternalInput")
    d1_d = nc.dram_tensor("dw1w", [128, 6 * 128], bf16, kind="ExternalInput")
    d2_d = nc.dram_tensor("dw2w", [128, 6 * 64], bf16, kind="ExternalInput")
    idb_d = nc.dram_tensor("idb", [128, 128], bf16, kind="ExternalInput")
    idf_d = nc.dram_tensor("idf", [128, 128], f32, kind="ExternalInput")
    pwT_d = nc.dram_tensor("pwT", [64, 64], f32, kind="ExternalInput")
    wfixT_d = nc.dram_tensor("wfixT", [64, 64], f32, kind="ExternalInput")
    pb_d = nc.dram_tensor("pb", [64, 1], f32, kind="ExternalInput")
    tq_d = nc.dram_tensor("tq", [64, 1], f32, kind="ExternalInput")
    em_d = nc.dram_tensor("emask", [128, 8], f32, kind="ExternalInput")
    blkm_d = nc.dram_tensor("blkm", [64, 64], f32, kind="ExternalInput")
    out_d = nc.dram_tensor("out", [C, ROWS * WIMG], f32, kind="ExternalOutput")
    if debug:
        gdbg_d = nc.dram_tensor("gdbg", [128, 128], f32, kind="ExternalOutput")
        adbg_d = nc.dram_tensor("adbg", [64, 8], f32, kind="ExternalOutput")
        vdbg_d = nc.dram_tensor("vdbg", [128, VROWS * WP], f32, kind="ExternalOutput")

    with tile.TileContext(nc) as tc:
        with (
            tc.tile_pool(name="const", bufs=1) as constp,
            tc.tile_pool(name="big", bufs=1) as bigp,
            tc.tile_pool(name="xwin", bufs=2) as xwp,
            tc.tile_pool(name="rows", bufs=3) as rowp,
            tc.tile_pool(name="glue", bufs=1) as gluep,
            tc.tile_pool(name="gwin", bufs=2) as gwp,
            tc.tile_pool(name="outs", bufs=2) as outp,
            tc.tile_pool(name="psg", bufs=1, space="PSUM") as psgp,
            tc.tile_pool(name="dram", bufs=1, space="DRAM") as dramp,
        ):
            # ---- constants into SBUF ----
            cw = constp.tile([128, 12 * 128], bf16)
            d1w = constp.tile([128, 6 * 128], bf16)
            d2w = constp.tile([128, 6 * 64], bf16)
            idb = constp.tile([128, 128], bf16)
            idf = constp.tile([128, 128], f32)
            pwT = constp.tile([64, 64], f32)
            wfixT = constp.tile([64, 64], f32)
            pb = constp.tile([64, 1], f32)
            tq = constp.tile([64, 1], f32)
            em = constp.tile([128, 8], f32)
            blkm = constp.tile([64, 64], f32)
            for t, d in (
                (cw, cw_d), (d1w, d1_d), (d2w, d2_d), (idb, idb_d), (idf, idf_d),
                (pwT, pwT_d), (wfixT, wfixT_d), (pb, pb_d), (tq, tq_d), (em, em_d),
                (blkm, blkm_d),
            ):
                nc.sync.dma_start(t[:], d.ap())

            # ---- persistent big buffers ----
            v2 = bigp.tile([128, (VROWS + 1) * WP], bf16)  # [v; v@WP], slot y = v row y-2, 1 slack row
            # zero pad columns once (cols 0 and 257 of each row, incl slack)
            v2v = v2[:].rearrange("p (r w) -> p r w", w=WP)
            nc.vector.memset(v2v[:, :, 0:1], 0.0)
            nc.vector.memset(v2v[:, :, 257:258], 0.0)

            G_ps = psgp.tile([128, 128], f32, tag="G")

            # ================= conv + gram phase =================
            gram_first = [True]

            def conv_window(w, psp):
                x2 = xwp.tile([128, 24 * WP], bf16, tag="xwin")
                # copy A: x rows [22w-3, 22w+21) = xp slab rows [22w, 22w+24)
                src0 = 22 * w * WP
                nc.sync.dma_start(x2[0:64, :], xp_d.ap()[:, src0 : src0 + 24 * WP])
                nc.sync.dma_start(x2[64:128, :], xp_d.ap()[:, src0 + WP : src0 + 25 * WP])
                for yv in range(22 * w - 2, 22 * w + 20):
                    slot = yv - (22 * w - 3)  # x-row slot of row yv in window
                    base = slot * WP + 1
                    do_qk = 0 <= yv < ROWS
                    passes = ([(0, True)] if do_qk else []) + [(6, False)]
                    for m0, is_qk in passes:
                        ps = psp.tile([128, 256], f32, tag="qkps" if is_qk else "vvps")
                        for i, (dy, dx) in enumerate(_BASES):
                            delta = dy * WP + dx
                            nc.tensor.matmul(
                                ps[:],
                                cw[:, 128 * (m0 + i) : 128 * (m0 + i + 1)],
                                x2[:, base + delta : base + delta + 256],
                                start=(i == 0),
                                stop=(i == 5),
                            )
                        if is_qk:
                            qkb = rowp.tile([128, 256], bf16, tag="qkb")
                            nc.scalar.copy(qkb[:], ps[:])
                            qkT = rowp.tile([128, 256], bf16, tag="qkT")
                            for h in range(2):
                                tps = psp.tile([128, 128], bf16, tag="tps")
                                nc.tensor.transpose(tps[:], qkb[:, 128 * h : 128 * h + 128], idb[:])
                                nc.vector.tensor_copy(qkT[:, 128 * h : 128 * h + 128], tps[:])
                            for h in range(2):
                                nc.tensor.matmul(
                                    G_ps[:],
                                    qkT[:, 128 * h : 128 * h + 128],
                                    qkT[:, 128 * h : 128 * h + 128],
                                    start=gram_first[0],
                                    stop=(yv == ROWS - 1 and h == 1),
                                )
                                gram_first[0] = False
                        else:
                            nc.scalar.copy(
                                v2[:, (yv + 2) * WP + 1 : (yv + 2) * WP + 257], ps[:]
                            )

            with tc.tile_pool(name="psA", bufs=2, space="PSUM") as psA:
                for w in range(NCW):
                    conv_window(w, psA)

            # zero out-of-image v rows (SAME padding for the pe branch)
            for ci, slot in ((3, 0), (4, 1), (5, 129), (6, 130), (7, 131)):
                nc.vector.tensor_scalar(
                    out=v2[:, slot * WP : (slot + 1) * WP],
                    in0=v2[:, slot * WP : (slot + 1) * WP],
                    scalar1=em[:, ci : ci + 1], scalar2=None, op0=ALU.mult,
                )

            # ================= gram allreduce + glue =================
            psB_cm = tc.tile_pool(name="psB", bufs=2, space="PSUM")
            psp = psB_cm.__enter__()
            psC_cm = tc.tile_pool(name="psC", bufs=1, space="PSUM")
            psc = psC_cm.__enter__()
            G_sb = gluep.tile([128, 128], f32)
            nc.scalar.copy(G_sb[:], G_ps[:])
            gin = dramp.tile([128, 128], f32)
            gout = dramp.tile([128, 128], f32)
            nc.sync.dma_start(gin[:], G_sb[:])
            nc.gpsimd.collective_compute(
                "AllReduce",
                mybir.AluOpType.add,
                replica_groups=[[0, 1], [2, 3], [4, 5], [6, 7]],
                ins=[gin[:].opt()],
                outs=[gout[:].opt()],
            )
            G2 = gluep.tile([128, 128], f32)
            nc.sync.dma_start(G2[:], gout[:])
            if debug:
                nc.sync.dma_start(gdbg_d.ap(), G2[:])
                vdbg = gluep.tile([128, VROWS * WP], f32)
                nc.vector.tensor_copy(vdbg[:], v2[:, : VROWS * WP])
                nc.sync.dma_start(vdbg_d.ap(), vdbg[:])

            # diag -> squared norms -> rn = 1/max(sqrt(ssq), eps)
            dd = gluep.tile([128, 128], f32)
            nc.vector.tensor_tensor(out=dd[:], in0=G2[:], in1=idf[:], op=ALU.mult)
            ssq = gluep.tile([128, 1], f32)
            nc.vector.tensor_reduce(ssq[:], dd[:], mybir.AxisListType.X, ALU.add)
            nrm = gluep.tile([128, 1], f32)
            nc.scalar.activation(nrm[:], ssq[:], AF.Sqrt)
            nc.vector.tensor_scalar_max(nrm[:], nrm[:], EPS)
            rn = gluep.tile([128, 1], f32)
            nc.vector.reciprocal(rn[:], nrm[:])
            # Gfull[c,d] = G2[c,d] * rn[c] * rn[d] via scale, transpose, scale, transpose
            Gs = gluep.tile([128, 128], f32)
            nc.vector.tensor_scalar(out=Gs[:], in0=G2[:], scalar1=rn[:], scalar2=None, op0=ALU.mult)
            t1 = psc.tile([128, 128], f32, tag="gt")
            nc.tensor.transpose(t1[:], Gs[:], idf[:])
            GsT = gluep.tile([128, 128], f32)
            nc.vector.tensor_scalar(out=GsT[:], in0=t1[:], scalar1=rn[:], scalar2=None, op0=ALU.mult)
            t2 = psc.tile([128, 128], f32, tag="gt")
            nc.tensor.transpose(t2[:], GsT[:], idf[:])
            Gfull = gluep.tile([128, 128], f32)
            nc.vector.tensor_copy(Gfull[:], t2[:])

            # per-head extraction * temperature -> S [64, 8]
            # masked blockdiag of the q-k quadrant, then strided reduce over groups
            msk = gluep.tile([64, 64], f32)
            nc.vector.tensor_tensor(out=msk[:], in0=Gfull[0:64, 64:128], in1=blkm[:], op=ALU.mult)
            S = gluep.tile([64, 8], f32)
            nc.vector.tensor_reduce(
                S[:], msk[:].rearrange("p (g d) -> p d g", d=8), mybir.AxisListType.X, ALU.add
            )
            nc.vector.tensor_scalar(out=S[:], in0=S[:], scalar1=tq[:], scalar2=None, op0=ALU.mult)
            # softmax along free dim (8)
            nmax = gluep.tile([64, 1], f32)
            nc.vector.tensor_reduce(nmax[:], S[:], mybir.AxisListType.X, ALU.max, negate=True)
            E = gluep.tile([64, 8], f32)
            nc.scalar.activation(E[:], S[:], AF.Exp, bias=nmax[:], scale=1.0)
            Z = gluep.tile([64, 1], f32)
            nc.vector.tensor_reduce(Z[:], E[:], mybir.AxisListType.X, ALU.add)
            rZ = gluep.tile([64, 1], f32)
            nc.vector.reciprocal(rZ[:], Z[:])
            A = gluep.tile([64, 8], f32)
            nc.vector.tensor_scalar(out=A[:], in0=E[:], scalar1=rZ[:], scalar2=None, op0=ALU.mult)
            if debug:
                nc.sync.dma_start(adbg_d.ap(), A[:])
            # blockdiag + fold into projection: WcT = (proj_w @ A_bd)^T + WfixT
            Arep = gluep.tile([64, 64], f32)
            nc.sync.dma_start(Arep[:], A[:].broadcast_to((64, 8, 8)).rearrange("p d g -> p g d"))
            Abd = gluep.tile([64, 64], f32)
            nc.vector.tensor_tensor(out=Abd[:], in0=Arep[:], in1=blkm[:], op=ALU.mult)
            wc_ps = psc.tile([64, 64], f32, tag="wc")
            nc.tensor.matmul(wc_ps[:], Abd[:], pwT[:], start=True, stop=True)
            WcT = gluep.tile([64, 64], bf16)
            nc.vector.tensor_tensor(out=WcT[:], in0=wc_ps[:], in1=wfixT[:], op=ALU.add)

            # ================= pe branch + attn tail =================
            def pe_window(pw):
                gsb = gwp.tile([128, 19 * WP], bf16, tag="gwin")
                gv = gsb[:].rearrange("p (r w) -> p r w", w=WP)
                nc.vector.memset(gv[:, :, 0:1], 0.0)
                nc.vector.memset(gv[:, :, 257:258], 0.0)
                yg0 = PW * pw - 1
                for yg in range(yg0, yg0 + 18):
                    slot = yg - yg0
                    gps = psp.tile([128, 256], f32, tag="gps")
                    vbase = (yg + 2) * WP + 1
                    for i in range(6):
                        dy, dx = _BASES[i]
                        delta = dy * WP + dx
                        nc.tensor.matmul(
                            gps[:],
                            d1w[:, 128 * i : 128 * i + 128],
                            v2[:, vbase + delta : vbase + delta + 256],
                            start=(i == 0),
                            stop=(i == 5),
                        )
                    nc.scalar.activation(
                        gsb[:, slot * WP + 1 : slot * WP + 257], gps[:], AF.Gelu
                    )
                # edge masks (rows outside the image must be zero)
                if pw == 0:
                    nc.vector.tensor_scalar(
                        out=gsb[:, 1:257], in0=gsb[:, 1:257],
                        scalar1=em[:, 0:1], scalar2=None, op0=ALU.mult,
                    )
                if pw == NPW - 1:
                    nc.vector.tensor_scalar(
                        out=gsb[:, 16 * WP + 1 : 16 * WP + 257],
                        in0=gsb[:, 16 * WP + 1 : 16 * WP + 257],
                        scalar1=em[:, 1:2], scalar2=None, op0=ALU.mult,
                    )
                    nc.vector.tensor_scalar(
                        out=gsb[:, 17 * WP + 1 : 17 * WP + 257],
                        in0=gsb[:, 17 * WP + 1 : 17 * WP + 257],
                        scalar1=em[:, 2:3], scalar2=None, op0=ALU.mult,
                    )
                osb = outp.tile([64, PW * 256], f32, tag="osb")
                for yo in range(PW * pw, PW * pw + PW):
                    oslot = yo - PW * pw
                    ops = psp.tile([64, 256], f32, tag="ops")
                    gbase = (yo - yg0) * WP + 1
                    for i in range(6):
                        dy, dx = _BASES[i]
                        delta = dy * WP + dx
                        nc.tensor.matmul(
                            ops[:],
                            d2w[:, 64 * i : 64 * i + 64],
                            gsb[:, gbase + delta : gbase + delta + 256],
                            start=(i == 0),
                            stop=False,
                        )
                    nc.tensor.matmul(
                        ops[:],
                        WcT[:],
                        v2[0:64, (yo + 2) * WP + 1 : (yo + 2) * WP + 257],
                        start=False,
                        stop=True,
                    )
                    nc.scalar.activation(
                        osb[:, oslot * 256 : oslot * 256 + 256], ops[:],
                        AF.Identity, bias=pb[:], scale=1.0,
                    )
                nc.sync.dma_start(
                    out_d.ap()[:, PW * pw * 256 : (PW * pw + PW) * 256], osb[:]
                )

            for pw in range(NPW):
                pe_window(pw)
            psC_cm.__exit__(None, None, None)
            psB_cm.__exit__(None, None, None)

    nc.compile()
    return nc


def _host_prep(inputs):
    x = np.asarray(inputs["x"], np.float32)
    qkv_w = np.asarray(inputs["qkv_w"], np.float32)
    dw_w = np.asarray(inputs["dw_w"], np.float32)
    proj_w = np.asarray(inputs["proj_w"], np.float32)[:, :, 0, 0]
    proj_b = np.asarray(inputs["proj_b"], np.float32)
    pos1_w = np.asarray(inputs["pos1_w"], np.float32)
    pos2_w = np.asarray(inputs["pos2_w"], np.float32)
    temperature = np.asarray(inputs["temperature"], np.float32).reshape(HEADS)
    rel_bias = np.asarray(inputs["rel_bias"], np.float32)

    cw = _build_conv_weights(qkv_w, dw_w).astype(ml_dtypes.bfloat16)
    d1w = _build_dw_weights(pos1_w, True).astype(ml_dtypes.bfloat16)
    d2w = _build_dw_weights(pos2_w, False).astype(ml_dtypes.bfloat16)
    idb = np.eye(128, dtype=ml_dtypes.bfloat16)
    idf = np.eye(128, dtype=np.float32)
    pwT = np.ascontiguousarray(proj_w.T)  # [m, o]
    ii = np.arange(CH)
    toep = rel_bias[ii[:, None] - ii[None, :] + CH - 1]  # [8, 8]
    wfix = proj_w @ np.kron(np.eye(HEADS, dtype=np.float32), toep)
    wfixT = np.ascontiguousarray(wfix.T.astype(np.float32))
    pb = proj_b.reshape(64, 1)
    tqv = np.repeat(temperature, CH).reshape(64, 1).astype(np.float32)

    blkm_host = np.zeros((64, 64), np.float32)
    for cc in range(64):
        g = cc // CH
        blkm_host[cc, CH * g : CH * g + CH] = 1.0

    # padded x: [B, C, XTOP+H+XBOT, WP]
    xp = np.zeros((B, C, XTOP + H + XBOT, WP), np.float32)
    xp[:, :, XTOP : XTOP + H, 1 : 1 + WIMG] = x.reshape(B, C, H, WIMG)

    in_maps = []
    for core in range(NCORES):
        s, half = core // 2, core % 2
        r0 = half * ROWS
        slab = np.ascontiguousarray(
            xp[s, :, r0 : r0 + XROWS, :].reshape(C, XROWS * WP)
        ).astype(ml_dtypes.bfloat16)
        em = np.ones((128, 8), np.float32)
        if half == 0:
            em[0:64, 0] = 0.0       # g row -1 (A half); B half holds g[0], keep
            em[:, 3] = 0.0          # v2 slot 0 (v[-2] / v[-1])
            em[0:64, 4] = 0.0       # v2 slot 1 A (v[-1]); B holds v[0], keep
        else:
            em[0:64, 2] = 0.0       # g row 128 (A half of slot 17)
            em[64:128, 1] = 0.0     # g row 128 (B half of slot 16)
            em[64:128, 2] = 0.0     # slot 17 B half (g row 129, garbage)
            em[64:128, 5] = 0.0     # v2 slot 129 B (v[128])
            em[:, 6] = 0.0          # v2 slot 130 (v[128] / v[129])
            em[:, 7] = 0.0          # v2 slot 131 (v[129] / v[130])
        in_maps.append(
            {
                "xp": slab, "cw": cw, "dw1w": d1w, "dw2w": d2w, "idb": idb,
                "idf": idf, "pwT": pwT, "wfixT": wfixT, "pb": pb, "tq": tqv,
                "emask": em, "blkm": blkm_host,
            }
        )
    return in_maps


def kernel(**inputs):
    from concourse import bass_utils

    if "prog" not in _cache:
        _cache["prog"] = _build_program()
    nc = _cache["prog"]
    in_maps = _host_prep(inputs)
    res = None
    last = None
    for _attempt in range(3):
        try:
            res = bass_utils.run_bass_kernel_spmd(
                nc, in_maps, core_ids=list(range(NCORES))
            )
            break
        except Exception as e:  # transient device-unrecoverable: reset + retry
            last = e
            try:
                import jax, time as _t

                jax.clear_backends()
                _t.sleep(3)
            except Exception:
                pass
    if res is None:
        raise last
    out = np.empty((B, C, H, WIMG), np.float32)
    for core in range(NCORES):
        s, half = core // 2, core % 2
        r0 = half * ROWS
        out[s, :, r0 : r0 + ROWS, :] = res.results[core]["out"].reshape(C, ROWS, WIMG)
    return out


# revision 12
# speedup vs baseline: 1.0465x; 1.0465x over previous
"""Trainium2 Bass kernel for SSTransformer channel-attention block.

Sharding: 8 cores; core c handles sample c//2, row-half c%2 (128 of 256 rows).
Per core, one fused Bass program computes:
  - fused qkv 1x1 conv + depthwise 3x3 (6 PSUM-accumulated matmuls per row over
    a duplicated row-shifted x layout [x; x@+1row] on 128 partitions)
  - Gram matrix of [q;k] (per-head q.k^T dots + squared norms) via PE transpose
  - tiny cross-core AllReduce (core pairs) of the 128x128 Gram
  - on-chip softmax + rel-bias fold into the projection weights
  - (attn@v + proj) and the positional branch dw3x3 -> GELU -> dw3x3, both
    accumulated in PSUM, + proj bias, DMA out.
"""

import sys

sys.path.insert(0, "/opt/trn_rl_repo")

import numpy as np
import ml_dtypes

HEADS = 8
C = 64
CH = 8
B = 4
H = 256
WIMG = 256
WP = 258          # padded row stride (1 zero col each side, shared)
NCORES = 8
ROWS = 128        # output rows per core
VR0 = -2          # v2 first row (relative to slab start)
VROWS = 132       # v2 rows: -2 .. 129
XTOP = 3          # zero-pad rows above image in padded x
XBOT = 4          # below
XROWS = 135       # x rows per core slab: r0-3 .. r0+131
CW = 22           # conv window height (v2 rows per window); 6 windows
NCW = 6
PW = 16           # pe window output rows; 8 windows
NPW = 8
EPS = 1e-12

_BASES = [(-1, -1), (-1, 0), (-1, 1), (1, -1), (1, 0), (1, 1)]  # (dy,dx) of base offsets

_cache = {}


def _build_conv_weights(qkv_w, dw_w):
    """12 lhsT matrices [128,128] f32, flattened to [128, 12*128].

    Matmul m (m<6: pass1 -> [q;k]; m>=6: pass2 -> [v; v@WP]) with base offset
    delta = dy*WP+dx reads partition p<64: x[p] at j+delta, p>=64: x[p-64] at
    j+delta+WP.
    """
    w1 = qkv_w[:, :, 0, 0]  # [192 oc, 64 ic]

    def tapw(oc0, dy, dx):
        # returns [64 ic, 64 oc] = lhsT block for tap (dy,dx), out channels oc0..oc0+64
        blk = w1[oc0 : oc0 + 64]  # [64 oc, 64 ic]
        d = dw_w[oc0 : oc0 + 64, 0, dy + 1, dx + 1]  # [64]
        return (blk * d[:, None]).T.astype(np.float32)

    mats = []
    for dy, dx in _BASES:
        m = np.zeros((128, 128), np.float32)
        # pass1: cols 0:64 = q (oc0=0), 64:128 = k (oc0=64)
        for ci, oc0 in ((0, 0), (64, 64)):
            m[0:64, ci : ci + 64] = tapw(oc0, dy, dx)          # A rows: tap (dy,dx)
            if dy == -1:
                m[64:128, ci : ci + 64] = tapw(oc0, 0, dx)     # B rows: tap (0,dx)
        mats.append(m)
    for dy, dx in _BASES:
        m = np.zeros((128, 128), np.float32)
        # pass2: cols 0:64 = v (oc0=128), 64:128 = v@WP
        m[0:64, 0:64] = tapw(128, dy, dx)
        if dy == -1:
            m[64:128, 0:64] = tapw(128, 0, dx)
            m[64:128, 64:128] = tapw(128, dy, dx)
        else:
            m[0:64, 64:128] = tapw(128, 0, dx)
            m[64:128, 64:128] = tapw(128, dy, dx)
        mats.append(m)
    return np.concatenate(mats, axis=1)  # [128, 12*128]


def _build_dw_weights(pos_w, out_shifted):
    """6 lhsT diag-block matrices for a depthwise 3x3 over [t; t@WP] input.

    out_shifted: if True output is [o; o@WP] (M=128), else just o (M=64).
    Returns [128, 6*M] float32 (cast to bf16 by caller).
    """
    M = 128 if out_shifted else 64

    def dtap(dy, dx):
        return np.diag(pos_w[:, 0, dy + 1, dx + 1]).astype(np.float32)

    mats = []
    for dy, dx in _BASES:
        m = np.zeros((128, M), np.float32)
        m[0:64, 0:64] = dtap(dy, dx)
        if dy == -1:
            m[64:128, 0:64] = dtap(0, dx)
            if out_shifted:
                m[64:128, 64:128] = dtap(dy, dx)
        else:
            if out_shifted:
                m[0:64, 64:128] = dtap(0, dx)
                m[64:128, 64:128] = dtap(dy, dx)
    # NOTE: for dy==+1 and not out_shifted, only A rows are used.
        mats.append(m)
    return np.concatenate(mats, axis=1)


def _build_program(debug=False):
    import concourse.bass as bass
    import concourse.bacc as bacc
    import concourse.mybir as mybir
    from concourse import tile

    dt = mybir.dt
    AF = mybir.ActivationFunctionType
    ALU = mybir.AluOpType
    f32, bf16 = dt.float32, dt.bfloat16

    nc = bacc.Bacc("TRN2", target_bir_lowering=False, debug=False, num_devices=NCORES)

    xp_d = nc.dram_tensor("xp", [C, XROWS * WP], bf16, kind="ExternalInput")
    cw_d = nc.dram_tensor("cw", [128, 12 * 128], bf16, kind="ExternalInput")
    d1_d = nc.dram_tensor("dw1w", [128, 6 * 128], bf16, kind="ExternalInput")
    d2_d = nc.dram_tensor("dw2w", [128, 6 * 64], bf16, kind="ExternalInput")
    idb_d = nc.dram_tensor("idb", [128, 128], bf16, kind="ExternalInput")
    idf_d = nc.dram_tensor("idf", [128, 128], f32, kind="ExternalInput")
    pwT_d = nc.dram_tensor("pwT", [64, 64], f32, kind="ExternalInput")
    wfixT_d = nc.dram_tensor("wfixT", [64, 64], f32, kind="ExternalInput")
    pb_d = nc.dram_tensor("pb", [64, 1], f32, kind="ExternalInput")
    tq_d = nc.dram_tensor("tq", [64, 1], f32, kind="ExternalInput")
    em_d = nc.dram_tensor("emask", [128, 8], f32, kind="ExternalInput")
    blkm_d = nc.dram_tensor("blkm", [64, 64], f32, kind="ExternalInput")
    out_d = nc.dram_tensor("out", [C, ROWS * WIMG], f32, kind="ExternalOutput")
    if debug:
        gdbg_d = nc.dram_tensor("gdbg", [128, 128], f32, kind="ExternalOutput")
        adbg_d = nc.dram_tensor("adbg", [64, 8], f32, kind="ExternalOutput")
        vdbg_d = nc.dram_tensor("vdbg", [128, VROWS * WP], f32, kind="ExternalOutput")

    with tile.TileContext(nc) as tc:
        with (
            tc.tile_pool(name="const", bufs=1) as constp,
            tc.tile_pool(name="big", bufs=1) as bigp,
            tc.tile_pool(name="xwin", bufs=3) as xwp,
            tc.tile_pool(name="rows", bufs=6) as rowp,
            tc.tile_pool(name="glue", bufs=1) as gluep,
            tc.tile_pool(name="gwin", bufs=2) as gwp,
            tc.tile_pool(name="outs", bufs=3) as outp,
            tc.tile_pool(name="psg", bufs=1, space="PSUM") as psgp,
            tc.tile_pool(name="dram", bufs=1, space="DRAM") as dramp,
        ):
            # ---- constants into SBUF ----
            cw = constp.tile([128, 12 * 128], bf16)
            d1w = constp.tile([128, 6 * 128], bf16)
            d2w = constp.tile([128, 6 * 64], bf16)
            idb = constp.tile([128, 128], bf16)
            idf = constp.tile([128, 128], f32)
            pwT = constp.tile([64, 64], f32)
            wfixT = constp.tile([64, 64], f32)
            pb = constp.tile([64, 1], f32)
            tq = constp.tile([64, 1], f32)
            em = constp.tile([128, 8], f32)
            blkm = constp.tile([64, 64], f32)
            for t, d in (
                (cw, cw_d), (d1w, d1_d), (d2w, d2_d), (idb, idb_d), (idf, idf_d),
                (pwT, pwT_d), (wfixT, wfixT_d), (pb, pb_d), (tq, tq_d), (em, em_d),
                (blkm, blkm_d),
            ):
                nc.sync.dma_start(t[:], d.ap())

            # ---- persistent big buffers ----
            v2 = bigp.tile([128, (VROWS + 1) * WP], bf16)  # [v; v@WP], slot y = v row y-2, 1 slack row
            # zero pad columns once (cols 0 and 257 of each row, incl slack)
            v2v = v2[:].rearrange("p (r w) -> p r w", w=WP)
            nc.vector.memset(v2v[:, :, 0:1], 0.0)
            nc.vector.memset(v2v[:, :, 257:258], 0.0)

            G_ps = psgp.tile([128, 128], f32, tag="G")

            # ================= conv + gram phase =================
            gram_first = [True]

            def conv_window(w, psp, psq):
                x2 = xwp.tile([128, 24 * WP], bf16, tag="xwin")
                # copy A: x rows [22w-3, 22w+21) = xp slab rows [22w, 22w+24)
                src0 = 22 * w * WP
                nc.sync.dma_start(x2[0:64, :], xp_d.ap()[:, src0 : src0 + 24 * WP])
                nc.sync.dma_start(x2[64:128, :], xp_d.ap()[:, src0 + WP : src0 + 25 * WP])
                for yv in range(22 * w - 2, 22 * w + 20):
                    slot = yv - (22 * w - 3)  # x-row slot of row yv in window
                    base = slot * WP + 1
                    do_qk = 0 <= yv < ROWS
                    passes = ([(0, True)] if do_qk else []) + [(6, False)]
                    for m0, is_qk in passes:
                        pool = psq if is_qk else psp
                        ps = pool.tile([128, 256], f32, tag="qkps" if is_qk else "vvps")
                        for i, (dy, dx) in enumerate(_BASES):
                            delta = dy * WP + dx
                            nc.tensor.matmul(
                                ps[:],
                                cw[:, 128 * (m0 + i) : 128 * (m0 + i + 1)],
                                x2[:, base + delta : base + delta + 256],
                                start=(i == 0),
                                stop=(i == 5),
                            )
                        if is_qk:
                            qkb = rowp.tile([128, 256], bf16, tag="qkb")
                            nc.scalar.copy(qkb[:], ps[:])
                            qkT = rowp.tile([128, 256], bf16, tag="qkT")
                            for h in range(2):
                                tps = psp.tile([128, 128], bf16, tag="tps")
                                nc.tensor.transpose(tps[:], qkb[:, 128 * h : 128 * h + 128], idb[:])
                                nc.vector.tensor_copy(qkT[:, 128 * h : 128 * h + 128], tps[:])
                            for h in range(2):
                                nc.tensor.matmul(
                                    G_ps[:],
                                    qkT[:, 128 * h : 128 * h + 128],
                                    qkT[:, 128 * h : 128 * h + 128],
                                    start=gram_first[0],
                                    stop=(yv == ROWS - 1 and h == 1),
                                )
                                gram_first[0] = False
                        else:
                            nc.scalar.copy(
                                v2[:, (yv + 2) * WP + 1 : (yv + 2) * WP + 257], ps[:]
                            )

            with (
                tc.tile_pool(name="psA", bufs=2, space="PSUM") as psA,
                tc.tile_pool(name="psQ", bufs=3, space="PSUM") as psQ,
            ):
                for w in range(NCW):
                    conv_window(w, psA, psQ)

            # zero out-of-image v rows (SAME padding for the pe branch)
            for ci, slot in ((3, 0), (4, 1), (5, 129), (6, 130), (7, 131)):
                nc.vector.tensor_scalar(
                    out=v2[:, slot * WP : (slot + 1) * WP],
                    in0=v2[:, slot * WP : (slot + 1) * WP],
                    scalar1=em[:, ci : ci + 1], scalar2=None, op0=ALU.mult,
                )

            # ================= gram allreduce + glue =================
            psB_cm = tc.tile_pool(name="psB", bufs=2, space="PSUM")
            psp = psB_cm.__enter__()
            psO_cm = tc.tile_pool(name="psO", bufs=3, space="PSUM")
            pso = psO_cm.__enter__()
            psC_cm = tc.tile_pool(name="psC", bufs=1, space="PSUM")
            psc = psC_cm.__enter__()
            G_sb = gluep.tile([128, 128], f32)
            nc.scalar.copy(G_sb[:], G_ps[:])
            gin = dramp.tile([128, 128], f32)
            gout = dramp.tile([128, 128], f32)
            nc.sync.dma_start(gin[:], G_sb[:])
            nc.gpsimd.collective_compute(
                "AllReduce",
                mybir.AluOpType.add,
                replica_groups=[[0, 1], [2, 3], [4, 5], [6, 7]],
                ins=[gin[:].opt()],
                outs=[gout[:].opt()],
            )
            G2 = gluep.tile([128, 128], f32)
            nc.sync.dma_start(G2[:], gout[:])
            if debug:
                nc.sync.dma_start(gdbg_d.ap(), G2[:])
                vdbg = gluep.tile([128, VROWS * WP], f32)
                nc.vector.tensor_copy(vdbg[:], v2[:, : VROWS * WP])
                nc.sync.dma_start(vdbg_d.ap(), vdbg[:])

            # diag -> squared norms -> rn = 1/max(sqrt(ssq), eps)
            dd = gluep.tile([128, 128], f32)
            nc.vector.tensor_tensor(out=dd[:], in0=G2[:], in1=idf[:], op=ALU.mult)
            ssq = gluep.tile([128, 1], f32)
            nc.vector.tensor_reduce(ssq[:], dd[:], mybir.AxisListType.X, ALU.add)
            nrm = gluep.tile([128, 1], f32)
            nc.scalar.activation(nrm[:], ssq[:], AF.Sqrt)
            nc.vector.tensor_scalar_max(nrm[:], nrm[:], EPS)
            rn = gluep.tile([128, 1], f32)
            nc.vector.reciprocal(rn[:], nrm[:])
            # Gfull[c,d] = G2[c,d] * rn[c] * rn[d] via scale, transpose, scale, transpose
            Gs = gluep.tile([128, 128], f32)
            nc.vector.tensor_scalar(out=Gs[:], in0=G2[:], scalar1=rn[:], scalar2=None, op0=ALU.mult)
            t1 = psc.tile([128, 128], f32, tag="gt")
            nc.tensor.transpose(t1[:], Gs[:], idf[:])
            GsT = gluep.tile([128, 128], f32)
            nc.vector.tensor_scalar(out=GsT[:], in0=t1[:], scalar1=rn[:], scalar2=None, op0=ALU.mult)
            t2 = psc.tile([128, 128], f32, tag="gt")
            nc.tensor.transpose(t2[:], GsT[:], idf[:])
            Gfull = gluep.tile([128, 128], f32)
            nc.vector.tensor_copy(Gfull[:], t2[:])

            # per-head extraction * temperature -> S [64, 8]
            # masked blockdiag of the q-k quadrant, then strided reduce over groups
            msk = gluep.tile([64, 64], f32)
            nc.vector.tensor_tensor(out=msk[:], in0=Gfull[0:64, 64:128], in1=blkm[:], op=ALU.mult)
            S = gluep.tile([64, 8], f32)
            nc.vector.tensor_reduce(
                S[:], msk[:].rearrange("p (g d) -> p d g", d=8), mybir.AxisListType.X, ALU.add
            )
            nc.vector.tensor_scalar(out=S[:], in0=S[:], scalar1=tq[:], scalar2=None, op0=ALU.mult)
            # softmax along free dim (8)
            nmax = gluep.tile([64, 1], f32)
            nc.vector.tensor_reduce(nmax[:], S[:], mybir.AxisListType.X, ALU.max, negate=True)
            E = gluep.tile([64, 8], f32)
            nc.scalar.activation(E[:], S[:], AF.Exp, bias=nmax[:], scale=1.0)
            Z = gluep.tile([64, 1], f32)
            nc.vector.tensor_reduce(Z[:], E[:], mybir.AxisListType.X, ALU.add)
            rZ = gluep.tile([64, 1], f32)
            nc.vector.reciprocal(rZ[:], Z[:])
            A = gluep.tile([64, 8], f32)
            nc.vector.tensor_scalar(out=A[:], in0=E[:], scalar1=rZ[:], scalar2=None, op0=ALU.mult)
            if debug:
                nc.sync.dma_start(adbg_d.ap(), A[:])
            # blockdiag + fold into projection: WcT = (proj_w @ A_bd)^T + WfixT
            Arep = gluep.tile([64, 64], f32)
            nc.sync.dma_start(Arep[:], A[:].broadcast_to((64, 8, 8)).rearrange("p d g -> p g d"))
            Abd = gluep.tile([64, 64], f32)
            nc.vector.tensor_tensor(out=Abd[:], in0=Arep[:], in1=blkm[:], op=ALU.mult)
            wc_ps = psc.tile([64, 64], f32, tag="wc")
            nc.tensor.matmul(wc_ps[:], Abd[:], pwT[:], start=True, stop=True)
            WcT = gluep.tile([64, 64], bf16)
            nc.vector.tensor_tensor(out=WcT[:], in0=wc_ps[:], in1=wfixT[:], op=ALU.add)

            # ================= pe branch + attn tail =================
            def pe_window(pw):
                gsb = gwp.tile([128, 19 * WP], bf16, tag="gwin")
                gv = gsb[:].rearrange("p (r w) -> p r w", w=WP)
                nc.vector.memset(gv[:, :, 0:1], 0.0)
                nc.vector.memset(gv[:, :, 257:258], 0.0)
                yg0 = PW * pw - 1
                for yg in range(yg0, yg0 + 18):
                    slot = yg - yg0
                    gps = psp.tile([128, 256], f32, tag="gps")
                    vbase = (yg + 2) * WP + 1
                    for i in range(6):
                        dy, dx = _BASES[i]
                        delta = dy * WP + dx
                        nc.tensor.matmul(
                            gps[:],
                            d1w[:, 128 * i : 128 * i + 128],
                            v2[:, vbase + delta : vbase + delta + 256],
                            start=(i == 0),
                            stop=(i == 5),
                        )
                    nc.scalar.activation(
                        gsb[:, slot * WP + 1 : slot * WP + 257], gps[:], AF.Gelu
                    )
                # edge masks (rows outside the image must be zero)
                if pw == 0:
                    nc.vector.tensor_scalar(
                        out=gsb[:, 1:257], in0=gsb[:, 1:257],
                        scalar1=em[:, 0:1], scalar2=None, op0=ALU.mult,
                    )
                if pw == NPW - 1:
                    nc.vector.tensor_scalar(
                        out=gsb[:, 16 * WP + 1 : 16 * WP + 257],
                        in0=gsb[:, 16 * WP + 1 : 16 * WP + 257],
                        scalar1=em[:, 1:2], scalar2=None, op0=ALU.mult,
                    )
                    nc.vector.tensor_scalar(
                        out=gsb[:, 17 * WP + 1 : 17 * WP + 257],
                        in0=gsb[:, 17 * WP + 1 : 17 * WP + 257],
                        scalar1=em[:, 2:3], scalar2=None, op0=ALU.mult,
                    )
                osb = outp.tile([64, PW * 256], f32, tag="osb")
                for yo in range(PW * pw, PW * pw + PW):
                    oslot = yo - PW * pw
                    ops = pso.tile([64, 256], f32, tag="ops")
                    gbase = (yo - yg0) * WP + 1
                    for i in range(6):
                        dy, dx = _BASES[i]
                        delta = dy * WP + dx
                        nc.tensor.matmul(
                            ops[:],
                            d2w[:, 64 * i : 64 * i + 64],
                            gsb[:, gbase + delta : gbase + delta + 256],
                            start=(i == 0),
                            stop=False,
                        )
                    nc.tensor.matmul(
                        ops[:],
                        WcT[:],
                        v2[0:64, (yo + 2) * WP + 1 : (yo + 2) * WP + 257],
                        start=False,
                        stop=True,
                    )
                    nc.scalar.activation(
                        osb[:, oslot * 256 : oslot * 256 + 256], ops[:],
                        AF.Identity, bias=pb[:], scale=1.0,
                    )
                nc.sync.dma_start(
                    out_d.ap()[:, PW * pw * 256 : (PW * pw + PW) * 256], osb[:]
                )

            for pw in range(NPW):
                pe_window(pw)
            psC_cm.__exit__(None, None, None)
            psO_cm.__exit__(None, None, None)
            psB_cm.__exit__(None, None, None)

    nc.compile()
    return nc


def _host_prep(inputs):
    x = np.asarray(inputs["x"], np.float32)
    qkv_w = np.asarray(inputs["qkv_w"], np.float32)
    dw_w = np.asarray(inputs["dw_w"], np.float32)
    proj_w = np.asarray(inputs["proj_w"], np.float32)[:, :, 0, 0]
    proj_b = np.asarray(inputs["proj_b"], np.float32)
    pos1_w = np.asarray(inputs["pos1_w"], np.float32)
    pos2_w = np.asarray(inputs["pos2_w"], np.float32)
    temperature = np.asarray(inputs["temperature"], np.float32).reshape(HEADS)
    rel_bias = np.asarray(inputs["rel_bias"], np.float32)

    cw = _build_conv_weights(qkv_w, dw_w).astype(ml_dtypes.bfloat16)
    d1w = _build_dw_weights(pos1_w, True).astype(ml_dtypes.bfloat16)
    d2w = _build_dw_weights(pos2_w, False).astype(ml_dtypes.bfloat16)
    idb = np.eye(128, dtype=ml_dtypes.bfloat16)
    idf = np.eye(128, dtype=np.float32)
    pwT = np.ascontiguousarray(proj_w.T)  # [m, o]
    ii = np.arange(CH)
    toep = rel_bias[ii[:, None] - ii[None, :] + CH - 1]  # [8, 8]
    wfix = proj_w @ np.kron(np.eye(HEADS, dtype=np.float32), toep)
    wfixT = np.ascontiguousarray(wfix.T.astype(np.float32))
    pb = proj_b.reshape(64, 1)
    tqv = np.repeat(temperature, CH).reshape(64, 1).astype(np.float32)

    blkm_host = np.zeros((64, 64), np.float32)
    for cc in range(64):
        g = cc // CH
        blkm_host[cc, CH * g : CH * g + CH] = 1.0

    # padded x: [B, C, XTOP+H+XBOT, WP]
    xp = np.zeros((B, C, XTOP + H + XBOT, WP), np.float32)
    xp[:, :, XTOP : XTOP + H, 1 : 1 + WIMG] = x.reshape(B, C, H, WIMG)

    in_maps = []
    for core in range(NCORES):
        s, half = core // 2, core % 2
        r0 = half * ROWS
        slab = np.ascontiguousarray(
            xp[s, :, r0 : r0 + XROWS, :].reshape(C, XROWS * WP)
        ).astype(ml_dtypes.bfloat16)
        em = np.ones((128, 8), np.float32)
        if half == 0:
            em[0:64, 0] = 0.0       # g row -1 (A half); B half holds g[0], keep
            em[:, 3] = 0.0          # v2 slot 0 (v[-2] / v[-1])
            em[0:64, 4] = 0.0       # v2 slot 1 A (v[-1]); B holds v[0], keep
        else:
            em[0:64, 2] = 0.0       # g row 128 (A half of slot 17)
            em[64:128, 1] = 0.0     # g row 128 (B half of slot 16)
            em[64:128, 2] = 0.0     # slot 17 B half (g row 129, garbage)
            em[64:128, 5] = 0.0     # v2 slot 129 B (v[128])
            em[:, 6] = 0.0          # v2 slot 130 (v[128] / v[129])
            em[:, 7] = 0.0          # v2 slot 131 (v[129] / v[130])
        in_maps.append(
            {
                "xp": slab, "cw": cw, "dw1w": d1w, "dw2w": d2w, "idb": idb,
                "idf": idf, "pwT": pwT, "wfixT": wfixT, "pb": pb, "tq": tqv,
                "emask": em, "blkm": blkm_host,
            }
        )
    return in_maps


def kernel(**inputs):
    from concourse import bass_utils

    if "prog" not in _cache:
        _cache["prog"] = _build_program()
    nc = _cache["prog"]
    in_maps = _host_prep(inputs)
    res = None
    last = None
    for _attempt in range(3):
        try:
            res = bass_utils.run_bass_kernel_spmd(
                nc, in_maps, core_ids=list(range(NCORES))
            )
            break
        except Exception as e:  # transient device-unrecoverable: reset + retry
            last = e
            try:
                import jax, time as _t

                jax.clear_backends()
                _t.sleep(3)
            except Exception:
                pass
    if res is None:
        raise last
    out = np.empty((B, C, H, WIMG), np.float32)
    for core in range(NCORES):
        s, half = core // 2, core % 2
        r0 = half * ROWS
        out[s, :, r0 : r0 + ROWS, :] = res.results[core]["out"].reshape(C, ROWS, WIMG)
    return out
